# revision 12
# baseline (speedup 1.0000x reference)
"""Trainium2 Bass kernel for nn_Actor (3 grouped conv1d blocks + dense + tanh).

Sharding: column-parallel across 8 cores. Core j owns input channels
{2j, 2j+1}; because every conv is grouped (depthwise x8 filters), that
slice owns contiguous channel blocks through the whole net:
  conv1 out-ch [16j,16j+16), conv2 out-ch [128j,128j+128),
  conv3 out-ch [1024j, 1024j+1024), and rows {l*8192 + ch} of W.
Each core computes a partial dense output [2, 64]; the host sums the 8
partials, adds bd and applies tanh.
"""

import numpy as np

import concourse.bass as bass
import concourse.tile as tile
from concourse import bacc
from concourse import mybir
from concourse.bass_utils import run_bass_kernel_spmd

# Problem constants (hardcoded; kernel.py must be self-contained)
B = 64
L = 128
C = 16
FILTERS = 8
K = 5
N_CORES = 8

L1 = L - K + 1      # 124  conv1 out length
L2 = L1 - K + 1     # 120  conv2 out length
L3 = L2 - K + 1     # 116  conv3 out length

C0 = C // N_CORES           # 2    input cols per core
C1 = C0 * FILTERS           # 16   conv1 out-ch per core
C2 = C1 * FILTERS           # 128  conv2 out-ch per core
C3 = C2 * FILTERS           # 1024 conv3 out-ch per core
G3 = C2 // C1               # 8    conv3 groups of 16 in-ch
BG = 8                      # batch groups for conv1 partition packing
B8 = B // BG                # 8

F32 = mybir.dt.float32

_CACHE = {}


def _build_nc():
    """Build the SPMD Bass program (same program on all 8 cores)."""
    nc = bacc.Bacc("TRN2", target_bir_lowering=False, debug=False)

    # DRAM parameters (per-core data supplied via in_maps).
    # Consolidated so each matmul waits on <=2 DMA queues (walrus LDW limit):
    #   a1: conv1 im2col;  cs: all conv stationaries+biases packed;
    #   wt: dense weight re-layout (loaded as 4 quarter-tiles).
    a1 = nc.declare_dram_parameter("a1", [80, L1 * B8], F32, isOutput=False)
    cs = nc.declare_dram_parameter("cs", [128, 1290], F32, isOutput=False)
    wt = nc.declare_dram_parameter("wt", [128, G3 * L3 * 2], F32, isOutput=False)
    out = nc.declare_dram_parameter("out", [2, B], F32, isOutput=True)

    NB1 = L1 * B8            # 992   conv1 free size (l1, b8) per bg
    NB2 = L2 * B             # 7680  conv2 free size (l2, b)
    NB3 = L3 * B             # 7424  conv3 free size (l3, b)

    with tile.TileContext(nc) as tc:
        with (
            tc.tile_pool(name="consts", bufs=1) as consts,
            tc.tile_pool(name="work", bufs=1) as work,
            tc.tile_pool(name="i3pool", bufs=2) as i3pool,
            tc.tile_pool(name="zpool", bufs=4) as zpool,
            tc.tile_pool(name="psum", bufs=4, space=bass.MemorySpace.PSUM) as psum,
            tc.tile_pool(name="psumd", bufs=1, space=bass.MemorySpace.PSUM) as psumd,
        ):
            # ---- load constants ----
            a1_t = consts.tile([80, NB1], F32)
            cs_t = consts.tile([128, 1290], F32)
            nc.sync.dma_start(a1_t[:], a1[:])
            nc.sync.dma_start(cs_t[:], cs[:])
            # cs layout: s1[0:128) b1[128] s2[129:257) b2[257] s3[258:1282) b3[1282:1290)
            WQ = G3 * L3 * 2 // 4  # 464 cols per wt quarter (2 conv3 groups)
            wt_ts = []
            for q in range(4):
                wq = consts.tile([128, WQ], F32, tag=f"wt{q}")
                nc.sync.dma_start(wq[:], wt[:, q * WQ:(q + 1) * WQ])
                wt_ts.append(wq)

            # ---- conv1: psum [128=(bg,c,f), (l1,b8)] in 2 chunks of 496 ----
            tmp1 = work.tile([128, NB1], F32)
            for ci in range(2):
                n = NB1 // 2  # 496
                p1 = psum.tile([128, n], F32, tag="pchunk")
                nc.tensor.matmul(p1[:], cs_t[0:80, 0:128], a1_t[:, ci * n:(ci + 1) * n],
                                 start=True, stop=True)
                nc.scalar.activation(tmp1[:, ci * n:(ci + 1) * n], p1[:],
                                     mybir.ActivationFunctionType.Relu,
                                     bias=cs_t[:, 128:129])

            # ---- reshape to x1r [16, (l1, b=bg*8+b8)] in one DMA ----
            # src iterates (cf, l1, bg, b8) over tmp1 [(bg,cf), (l1,b8)];
            # dst same order over x1r [cf, (l1, bg, b8)].
            x1r = work.tile([C1, L1 * B], F32)
            x1r_3d = x1r[:].rearrange("c (l b) -> c l b", l=L1)
            tmp1_4d = tmp1[:].rearrange("(bg c) (l b8) -> bg c l b8", bg=BG, l=L1)
            for bg in range(BG):
                nc.sync.dma_start(x1r_3d[:, :, bg * B8:(bg + 1) * B8],
                                  tmp1_4d[bg])

            # ---- conv2 im2col (one DMA): i2[k*16+c, (l2,b)] = x1r[c, (l2+k,b)] ----
            i2 = work.tile([80, NB2], F32)
            xb = x1r[0:C1, 0:NB2]
            src = bass.AP(xb.tensor, xb.offset,
                          [xb.ap[0], [B, K], xb.ap[1]])
            nc.sync.dma_start(i2[:], src)

            # ---- conv2 + relu -> x2r [128, (l2, b)] ----
            x2r = work.tile([C2, NB2], F32)
            for ci in range(NB2 // 512):  # 15 chunks
                p2 = psum.tile([128, 512], F32, tag="pchunk")
                nc.tensor.matmul(p2[:], cs_t[0:80, 129:257], i2[:, ci * 512:(ci + 1) * 512],
                                 start=True, stop=True)
                dst = x2r[:, ci * 512:(ci + 1) * 512]
                if ci % 2 == 0:
                    nc.scalar.activation(dst, p2[:],
                                         mybir.ActivationFunctionType.Relu,
                                         bias=cs_t[:, 257:258])
                else:
                    nc.vector.tensor_scalar(dst, p2[:], cs_t[:, 257:258], 0.0,
                                            mybir.AluOpType.add,
                                            mybir.AluOpType.max)

            # ---- conv3 (8 groups) + fused dense ----
            pd = psumd.tile([2, B], F32)
            # conv3 chunking: 7424 = 14*512 + 256
            chunks = [(i * 512, 512) for i in range(14)] + [(14 * 512, 256)]
            n_dense = 0
            for g in range(G3):
                i3 = i3pool.tile([80, NB3], F32, tag="i3")
                x2b = x2r[g * C1:(g + 1) * C1, 0:NB3]
                src = bass.AP(x2b.tensor, x2b.offset,
                              [x2b.ap[0], [B, K], x2b.ap[1]])
                nc.sync.dma_start(i3[:], src)
                for ci, (off, n) in enumerate(chunks):
                    p3 = psum.tile([128, n], F32, tag="pchunk")
                    nc.tensor.matmul(p3[:], cs_t[0:80, 258 + g * 128:258 + (g + 1) * 128],
                                     i3[:, off:off + n], start=True, stop=True)
                    z = zpool.tile([128, n], F32, tag="z")
                    if ci % 2 == 0:
                        nc.scalar.activation(z[:], p3[:],
                                             mybir.ActivationFunctionType.Relu,
                                             bias=cs_t[:, 1282 + g:1283 + g])
                    else:
                        nc.vector.tensor_scalar(z[:], p3[:], cs_t[:, 1282 + g:1283 + g],
                                                0.0, mybir.AluOpType.add,
                                                mybir.AluOpType.max)
                    for li in range(n // B):  # dense MMs, l' = off//B + li
                        lp = off // B + li
                        wcol = 232 * (g % 2) + lp * 2
                        nc.tensor.matmul(pd[:], wt_ts[g // 2][:, wcol:wcol + 2],
                                         z[:, li * B:(li + 1) * B],
                                         start=(n_dense == 0),
                                         stop=(n_dense == G3 * L3 - 1))
                        n_dense += 1

            # ---- write partial out ----
            out_t = work.tile([2, B], F32)
            nc.vector.tensor_copy(out_t[:], pd[:])
            nc.sync.dma_start(out[:], out_t[:])

    nc.compile()
    return nc


def _shard_inputs(state, k1, b1, k2, b2, k3, b3, W, bd):
    """Host-side: build per-core input maps (layout only, no math)."""
    state = np.asarray(state, dtype=np.float32)
    k1 = np.asarray(k1, np.float32); b1 = np.asarray(b1, np.float32)
    k2 = np.asarray(k2, np.float32); b2 = np.asarray(b2, np.float32)
    k3 = np.asarray(k3, np.float32); b3 = np.asarray(b3, np.float32)
    W = np.asarray(W, np.float32)
    W3 = W.reshape(L3, C3 * N_CORES, 2)

    in_maps = []
    for j in range(N_CORES):
        x0 = state[:, :, C0 * j:C0 * (j + 1)]  # [B, L, 2]

        # conv1 im2col [80=(bg,k,c), (l1, b8)]
        a1 = np.zeros((80, L1 * B8), np.float32)
        for bg in range(BG):
            for k in range(K):
                for c in range(C0):
                    # rows within bg block: k*2 + c
                    a1[bg * 10 + k * C0 + c] = (
                        x0[bg * B8:(bg + 1) * B8, k:k + L1, c].T.reshape(-1))
        # conv1 stationary blockdiag [80, 128=(bg,c,f)]
        s1 = np.zeros((80, 128), np.float32)
        for bg in range(BG):
            for c in range(C0):
                for k in range(K):
                    for f in range(FILTERS):
                        s1[bg * 10 + k * C0 + c,
                           bg * C1 + c * FILTERS + f] = k1[k, 0, (C0 * j + c) * FILTERS + f]
        b1p = np.tile(b1[C1 * j:C1 * (j + 1)], BG).reshape(128, 1).astype(np.float32)

        # conv2 stationary [80=(k,c2), 128=(c2,f)]
        s2 = np.zeros((80, 128), np.float32)
        for k in range(K):
            for c in range(C1):
                for f in range(FILTERS):
                    s2[c * K + k, c * FILTERS + f] = k2[k, 0, (C1 * j + c) * FILTERS + f]
        b2p = b2[C2 * j:C2 * (j + 1)].reshape(128, 1).astype(np.float32)

        # conv3 stationaries [80=(k,c), G3*128]
        s3 = np.zeros((80, G3 * 128), np.float32)
        for g in range(G3):
            for k in range(K):
                for c in range(C1):
                    for f in range(FILTERS):
                        s3[c * K + k, g * 128 + c * FILTERS + f] = (
                            k3[k, 0, (C2 * j + C1 * g + c) * FILTERS + f])
        b3p = np.empty((128, G3), np.float32)
        for g in range(G3):
            b3p[:, g] = b3[(C2 * j + C1 * g) * FILTERS:
                           (C2 * j + C1 * g) * FILTERS + 128]

        # dense weight re-layout [128=p(ch within block), (g, l', a)]
        wt = np.empty((128, G3, L3, 2), np.float32)
        for g in range(G3):
            # W3[l', 1024j + 128g + p, a] -> wt[p, g, l', a]
            wt[:, g] = W3[:, C3 * j + 128 * g:C3 * j + 128 * (g + 1), :].transpose(1, 0, 2)
        wt = wt.reshape(128, G3 * L3 * 2)

        cs = np.zeros((128, 1290), np.float32)
        cs[0:80, 0:128] = s1
        cs[:, 128] = b1p[:, 0]
        cs[0:80, 129:257] = s2
        cs[:, 257] = b2p[:, 0]
        cs[0:80, 258:1282] = s3
        cs[:, 1282:1290] = b3p

        in_maps.append({"a1": a1, "cs": cs, "wt": wt})
    return in_maps


def kernel(state, k1, b1, k2, b2, k3, b3, W, bd, **run_kwargs):
    if "nc" not in _CACHE:
        _CACHE["nc"] = _build_nc()
    nc = _CACHE["nc"]
    in_maps = _shard_inputs(state, k1, b1, k2, b2, k3, b3, W, bd)
    res = run_bass_kernel_spmd(nc, in_maps, list(range(N_CORES)), **run_kwargs)
    partials = np.stack([res.results[j]["out"] for j in range(N_CORES)])  # [8, 2, B]
    total = partials.sum(axis=0).T + np.asarray(bd, np.float32)  # [B, 2]
    out = np.tanh(total).astype(np.float32)
    if run_kwargs.get("trace"):
        _CACHE["last_result"] = res
    return out
